# revision 1
# baseline (speedup 1.0000x reference)
"""CrossWinAttention Trainium2 kernel, v2.

Full inputs in, full output out. Shards the 128 windows (b=2 x x=8 x y=8)
across 8 NeuronCores (16 windows each), runs a Bass/Tile kernel per core.

Structure (per pair of windows = one y-pair of one (b,x) slot), emitted
software-pipelined (frontend/midend/backend skewed across pairs):
  - 6 merged input DMAs ([128,(n,w1,w2h),(w2l,d)] layout, one per tensor
    per window; token order within a window is permuted consistently,
    un-permuted at the final mean-write).
  - LN: 12 bn_stats + 5 batched combine ops + Ln/Exp rstd + 12 DVE
    applies + 12 PE transposes + 3 DVE PSUM->SBUF copies.
  - projections: K (1 matmul, pair-wide), V (4); embedded Q per window
    (4 matmuls) so scores contract the full 128 partitions.
  - scores: 2 matmuls per (window, k-chunk) (N=512) + exp on ACT.
  - colsum + PV: per (head, k-chunk) matmuls, head-aligned via
    column-offset tile_position (row-offset tile positions crash the HW).
  - normalize: DVE reciprocal + multiply on head-aligned [128,256] tiles.
  - mean over n on gpsimd with permuted final write; output proj; 1 DMA.
"""

import numpy as np
import ml_dtypes

import concourse.bass as bass
import concourse.tile as tile
from concourse import mybir
from concourse.bass_utils import run_bass_kernel_spmd

F32 = mybir.dt.float32
BF16 = mybir.dt.bfloat16
FP8 = mybir.dt.float8e4
AF = mybir.ActivationFunctionType
ALU = mybir.AluOpType
DR = mybir.MatmulPerfMode.DoubleRow

HEADS, DH, D = 4, 32, 128
INNER = HEADS * DH
EPS = 1e-5
SCALE = DH ** -0.5
NPAIR = 2
NY = 8

# tuning knobs
USE_DR = False       # fp8 DoubleRow colsum+PV
ACT_DMA = True       # issue half the input DMAs from the ACT HWDGE ring
POOL_APPLY = False   # LN apply on gpsimd (TensorScalarPtr may lack Q7 ucode)

_BUILD_CACHE = {}


def _split_multi_waits(module):
    """This container's walrus rejects instructions with >1 sync wait. Engines
    execute in order, so hoist extra waits onto preceding NoOps."""
    import copy
    import bass_rust

    for function in module.functions:
        new_blocks = []
        for block in function.blocks:
            insts = []
            for inst in block.instructions:
                si = getattr(inst, "sync_info", None)
                waits = list(si.on_wait) if si is not None and si.on_wait else []
                if len(waits) > 1:
                    for k, w in enumerate(waits[:-1]):
                        insts.append(mybir.InstNoOp(
                            name=f"{inst.name}-w{k}",
                            engine=inst.engine,
                            ins=[], outs=[],
                            sync_info=bass_rust.SyncInfo(
                                on_wait=[w], on_update=[]),
                        ))
                    inst = copy.replace(
                        inst,
                        sync_info=bass_rust.SyncInfo(
                            on_wait=[waits[-1]], on_update=list(si.on_update)),
                    )
                insts.append(inst)
            new_blocks.append(copy.replace(block, instructions=insts))
        function.blocks.clear()
        for nb in new_blocks:
            function.blocks.append(nb)
    return module


def _build_program(add_cq, add_ck, add_cv, add_bp, split=True, loop=1,
                   use_dr=USE_DR):
    key = (add_cq, add_ck, add_cv, add_bp, split, loop, use_dr, ACT_DMA,
           POOL_APPLY)
    if key in _BUILD_CACHE:
        return _BUILD_CACHE[key]

    ED = FP8 if use_dr else BF16

    nc = bass.Bass()

    xq_d = nc.dram_tensor("xq", [NPAIR, 4, NY, 8, 8, D], F32, kind="ExternalInput")
    xk_d = nc.dram_tensor("xk", [NPAIR, 4, NY, 8, 8, D], F32, kind="ExternalInput")
    xv_d = nc.dram_tensor("xv", [NPAIR, 4, NY, 8, 8, D], F32, kind="ExternalInput")
    wqp_d = nc.dram_tensor("wqp", [D, 4, INNER], BF16, kind="ExternalInput")
    wk_d = nc.dram_tensor("wk", [D, INNER], BF16, kind="ExternalInput")
    wv_d = nc.dram_tensor("wv", [D, INNER], BF16, kind="ExternalInput")
    wp_d = nc.dram_tensor("wp", [INNER, D], BF16, kind="ExternalInput")
    ident_d = nc.dram_tensor("ident", [128, 128], BF16, kind="ExternalInput")
    cq_d = nc.dram_tensor("cq", [INNER, 1], F32, kind="ExternalInput") if add_cq else None
    ck_d = nc.dram_tensor("ck", [INNER, 1], F32, kind="ExternalInput") if add_ck else None
    cv_d = nc.dram_tensor("cv", [INNER, 1], F32, kind="ExternalInput") if add_cv else None
    bp_d = nc.dram_tensor("bpb", [128, D], F32, kind="ExternalInput") if add_bp else None
    z_d = nc.dram_tensor("z", [NPAIR, NY, 8, 8, D], F32, kind="ExternalOutput")

    with tile.TileContext(nc) as tc:
        with (
            tc.tile_pool(name="const", bufs=1) as constp,
            tc.tile_pool(name="xin", bufs=3) as xin,
            tc.tile_pool(name="xln", bufs=3) as xlnp,
            tc.tile_pool(name="stat", bufs=3) as statp,
            tc.tile_pool(name="xt", bufs=3) as xtp,
            tc.tile_pool(name="qkv", bufs=3) as qkvp,
            tc.tile_pool(name="exps", bufs=3) as expp,
            tc.tile_pool(name="sml", bufs=4) as smlp,
            tc.tile_pool(name="ps_xt", bufs=1, space="PSUM") as ps_xt,
            tc.tile_pool(name="ps_q", bufs=1, space="PSUM") as ps_q,
            tc.tile_pool(name="ps_kv", bufs=1, space="PSUM") as ps_kv,
            tc.tile_pool(name="ps_s", bufs=1, space="PSUM") as ps_s,
            tc.tile_pool(name="ps_sa", bufs=2, space="PSUM") as ps_sa,
        ):
            wqp_sb = constp.tile([D, 4, INNER], BF16, tag="wqp")
            wk_sb = constp.tile([D, INNER], BF16, tag="wk")
            wv_sb = constp.tile([D, INNER], BF16, tag="wv")
            wp_sb = constp.tile([INNER, D], BF16, tag="wp")
            ident = constp.tile([128, 128], BF16, tag="ident")
            eps_t = constp.tile([128, 1], F32, tag="eps")
            nc.vector.memset(eps_t, EPS)
            onesdr = constp.tile([128, 2, 32], ED, tag="onesdr")
            nc.vector.memset(onesdr, 1.0)
            nc.sync.dma_start(out=wqp_sb, in_=wqp_d[:, :, :])
            nc.sync.dma_start(out=wk_sb, in_=wk_d[:, :])
            nc.sync.dma_start(out=wv_sb, in_=wv_d[:, :])
            nc.sync.dma_start(out=wp_sb, in_=wp_d[:, :])
            nc.sync.dma_start(out=ident, in_=ident_d[:, :])
            cq_sb = ck_sb = cv_sb = bp_sb = None
            if add_cq:
                cq_sb = constp.tile([INNER, 1], F32, tag="cq")
                nc.sync.dma_start(out=cq_sb, in_=cq_d[:, :])
            if add_ck:
                ck_sb = constp.tile([INNER, 1], F32, tag="ck")
                nc.sync.dma_start(out=ck_sb, in_=ck_d[:, :])
            if add_cv:
                cv_sb = constp.tile([INNER, 1], F32, tag="cv")
                nc.sync.dma_start(out=cv_sb, in_=cv_d[:, :])
            if add_bp:
                bp_sb = constp.tile([128, D], F32, tag="bpb")
                nc.sync.dma_start(out=bp_sb, in_=bp_d[:, :])

            def frontend(g):
                p, y0 = g // 4, 2 * (g % 4)

                # ---- input DMAs: 1 per (tensor, window), split SP/ACT ----
                # xa[:, 2t+wi, :]: [128 part=(n,w1,w2h), 256 free=(w2l,d)]
                xa = xin.tile([128, 6, 2, 128], F32, tag="xa")
                for t, src_d in enumerate((xq_d, xk_d, xv_d)):
                    for wi in range(2):
                        s = 2 * t + wi
                        sap = bass.AP(
                            tensor=src_d,
                            offset=p * 262144 + (y0 + wi) * 8192,
                            ap=[[65536, 4], [1, 8192]],
                        )
                        eng = nc.sync if (s % 2 == 0 or not ACT_DMA) else nc.scalar
                        eng.dma_start(out=xa[:, s, :, :], in_=sap)

                # ---- LN stats (batched) ----
                st = statp.tile([128, 6, 2, 6], F32, tag="st")
                for s in range(6):
                    for l in range(2):
                        nc.vector.bn_stats(out=st[:, s, l, :],
                                           in_=xa[:, s, l, :])
                me, mo = st[:, :, :, 1], st[:, :, :, 4]
                m2e, m2o = st[:, :, :, 2], st[:, :, :, 5]
                mu = statp.tile([128, 6, 2], F32, tag="mu")
                ssum = statp.tile([128, 6, 2], F32, tag="ssum")
                nc.vector.tensor_add(out=ssum, in0=me, in1=mo)
                nc.vector.tensor_scalar_mul(out=mu, in0=ssum, scalar1=0.5)
                dmean = statp.tile([128, 6, 2], F32, tag="dmean")
                nc.vector.tensor_sub(out=dmean, in0=me, in1=mo)
                d2 = statp.tile([128, 6, 2], F32, tag="d2")
                nc.vector.tensor_mul(out=d2, in0=dmean, in1=dmean)
                sm2 = statp.tile([128, 6, 2], F32, tag="sm2")
                nc.vector.tensor_add(out=sm2, in0=m2e, in1=m2o)
                v128 = statp.tile([128, 6, 2], F32, tag="v128")
                nc.vector.scalar_tensor_tensor(
                    out=v128, in0=d2, scalar=32.0, in1=sm2,
                    op0=ALU.mult, op1=ALU.add)
                lnv = statp.tile([128, 6, 2], F32, tag="lnv")
                nc.scalar.activation(out=lnv, in_=v128, func=AF.Ln,
                                     bias=eps_t[:, 0:1], scale=1.0 / 128.0)
                rstd = statp.tile([128, 6, 2], F32, tag="rstd")
                nc.scalar.activation(out=rstd, in_=lnv, func=AF.Exp,
                                     scale=-0.5)

                # ---- LN apply (gpsimd) + PE transpose, per tensor ----
                xl = xlnp.tile([128, 6, 2, 128], BF16, tag="xl")
                xT = xtp.tile([128, 6, 2, 128], BF16, tag="xT")
                for t in range(3):
                    psXT = ps_xt.tile([128, 2, 2, 128], BF16, tag="psxt")
                    for wi in range(2):
                        s = 2 * t + wi
                        for l in range(2):
                            eng = nc.gpsimd if POOL_APPLY else nc.vector
                            eng.tensor_scalar(
                                out=xl[:, s, l, :], in0=xa[:, s, l, :],
                                scalar1=mu[:, s, l:l + 1],
                                scalar2=rstd[:, s, l:l + 1],
                                op0=ALU.subtract, op1=ALU.mult)
                            nc.tensor.transpose(
                                out=psXT[:, wi, l, :], in_=xl[:, s, l, :],
                                identity=ident)
                    nc.vector.tensor_copy(out=xT[:, 2 * t:2 * t + 2], in_=psXT)
                return {"xT": xT}

            def midend(g, stt):
                xT = stt["xT"]
                # ---- projections (K/V; Q per-window in backend) ----
                psK = ps_kv.tile([128, 512], F32, tag="pskv")
                nc.tensor.matmul(out=psK, lhsT=wk_sb,
                                 rhs=xT[:, 2:4], start=True, stop=True)
                # K^T -> SBUF (ACT, + ck bias), layout [128,(wi,kc),128]
                qk = qkvp.tile([128, 2, 2, 128], BF16, tag="qk")
                qk_in = psK.rearrange("p (a b c) -> p a b c", a=2, b=2)
                if add_ck:
                    nc.scalar.activation(out=qk, in_=qk_in,
                                         func=AF.Copy, bias=ck_sb[:, 0:1])
                else:
                    nc.scalar.copy(out=qk, in_=qk_in)

                psV = ps_kv.tile([128, 512], F32, tag="pskv")
                for c4 in range(4):
                    wi, l = c4 // 2, c4 % 2
                    nc.tensor.matmul(
                        out=psV[:, 128 * c4:128 * (c4 + 1)],
                        lhsT=xT[:, 4 + wi, l, :], rhs=wv_sb,
                        start=True, stop=True)
                # V -> SBUF (DVE; gpsimd cannot read PSUM): vt[:, wi, h, l, dh]
                vt = qkvp.tile([128, 2, 4, 2, 32], ED, tag="vt")
                for wi in range(2):
                    nc.vector.tensor_copy(
                        out=vt[:, wi],
                        in_=psV[:, 256 * wi:256 * (wi + 1)].rearrange(
                            "p (l h dh) -> p h l dh", l=2, h=4))
                stt.update(qk=qk, vt=vt, xT=xT)

            def backend(g, stt):
                p, y0 = g // 4, 2 * (g % 4)
                qk, vt, xT = stt["qk"], stt["vt"], stt["xT"]
                zt = smlp.tile([128, D], F32, tag="zt")
                psZ = ps_kv.tile([128, D], F32, tag="pskv")
                for wi in range(2):
                    # ---- embedded Q projection for this window ----
                    psQe = ps_q.tile([128, 4, 256], F32, tag="psqe")
                    for h in range(HEADS):
                        nc.tensor.matmul(out=psQe[:, h, :],
                                         lhsT=wqp_sb[:, h, :],
                                         rhs=xT[:, wi, :, :],
                                         start=True, stop=True)
                    qpad = qkvp.tile([128, 4, 256], BF16, tag="qpad")
                    if add_cq:
                        nc.scalar.activation(out=qpad, in_=psQe,
                                             func=AF.Copy, bias=cq_sb[:, 0:1])
                    elif wi == 0:
                        nc.vector.tensor_copy(out=qpad, in_=psQe)
                    else:
                        nc.scalar.copy(out=qpad, in_=psQe)

                    # ---- scores (full-128 embedded contraction) + exp ----
                    expw = expp.tile([128, 2, 2, 512], ED, tag="expw")
                    for kc in range(2):
                        psS = ps_s.tile([128, 2, 512], F32, tag="pss")
                        for hp in range(2):
                            nc.tensor.matmul(
                                out=psS[:, hp, :],
                                lhsT=qk[:, wi, kc, :],
                                rhs=qpad[:, 2 * hp:2 * (hp + 1), :],
                                start=True, stop=True)
                            nc.scalar.activation(out=expw[:, kc, hp, :],
                                                 in_=psS[:, hp, :],
                                                 func=AF.Exp, scale=SCALE)

                    # ---- colsum + PV ----
                    psSA = ps_sa.tile([128, 2, 256], F32, tag="pssa")
                    psSum, psA = psSA[:, 0, :], psSA[:, 1, :]
                    if use_dr:
                        for h in range(HEADS):
                            rhs = expw[:, :, h // 2,
                                       256 * (h % 2):256 * (h % 2 + 1)]
                            nc.tensor.matmul(
                                out=psSum[32 * h:32 * (h + 1), :],
                                lhsT=onesdr, rhs=rhs, start=True, stop=True,
                                perf_mode=DR, tile_position=(0, 32 * h),
                                skip_group_check=True)
                            nc.tensor.matmul(
                                out=psA[32 * h:32 * (h + 1), :],
                                lhsT=vt[:, wi, h], rhs=rhs,
                                start=True, stop=True,
                                perf_mode=DR, tile_position=(0, 32 * h),
                                skip_group_check=True)
                    else:
                        # each accumulation group must complete before the
                        # next one starts at the same tile position
                        for h in range(HEADS):
                            for kc in range(2):
                                nc.tensor.matmul(
                                    out=psSum[32 * h:32 * (h + 1), :],
                                    lhsT=onesdr[:, 0, :],
                                    rhs=expw[:, kc, h // 2,
                                             256 * (h % 2):256 * (h % 2 + 1)],
                                    start=(kc == 0), stop=(kc == 1),
                                    tile_position=(0, 32 * h),
                                    skip_group_check=True)
                            for kc in range(2):
                                nc.tensor.matmul(
                                    out=psA[32 * h:32 * (h + 1), :],
                                    lhsT=vt[:, wi, h, kc, :],
                                    rhs=expw[:, kc, h // 2,
                                             256 * (h % 2):256 * (h % 2 + 1)],
                                    start=(kc == 0), stop=(kc == 1),
                                    tile_position=(0, 32 * h),
                                    skip_group_check=True)

                    # ---- normalize ----
                    rcp = smlp.tile([128, 256], F32, tag="rcp")
                    nc.vector.reciprocal(out=rcp, in_=psSum)
                    aT = smlp.tile([128, 256], BF16, tag="aT")
                    nc.vector.tensor_tensor(out=aT, in0=psA, in1=rcp,
                                            op=ALU.mult)
                    if add_cv:
                        nc.vector.tensor_scalar(out=aT, in0=aT,
                                                scalar1=cv_sb[:, 0:1],
                                                scalar2=None, op0=ALU.add)

                    # ---- mean over n (gpsimd), permuted final write ----
                    # aT cols = (w2l2, n4, w1w2h 32)
                    av = aT.rearrange("p (a n b) -> p a n b", a=2, n=4)
                    t1 = smlp.tile([128, 2, 8, 4], BF16, tag="t1")
                    t2 = smlp.tile([128, 2, 8, 4], BF16, tag="t2")
                    nc.gpsimd.tensor_tensor(
                        out=t1.rearrange("p a w u -> p a (w u)"),
                        in0=av[:, :, 0, :], in1=av[:, :, 1, :], op=ALU.add)
                    nc.gpsimd.tensor_tensor(
                        out=t2.rearrange("p a w u -> p a (w u)"),
                        in0=av[:, :, 2, :], in1=av[:, :, 3, :], op=ALU.add)
                    abar = smlp.tile([128, 64], BF16, tag="abar")
                    # write (w2l,w1,w2h)-ordered sums into (w1,w2h,w2l) slots
                    abar_perm = abar.rearrange(
                        "p (w u a) -> p a w u", w=8, u=4)
                    nc.gpsimd.tensor_tensor(out=abar_perm, in0=t1, in1=t2,
                                            op=ALU.add)

                    # ---- output projection ----
                    nc.tensor.matmul(out=psZ[64 * wi:64 * (wi + 1), :],
                                     lhsT=abar, rhs=wp_sb, start=True,
                                     stop=True,
                                     tile_position=((0, 64) if wi else None))
                    if add_bp:
                        nc.vector.tensor_tensor(
                            out=zt[64 * wi:64 * (wi + 1), :],
                            in0=psZ[64 * wi:64 * (wi + 1), :],
                            in1=bp_sb[64 * wi:64 * (wi + 1), :],
                            op=ALU.add)
                    else:
                        nc.vector.tensor_copy(
                            out=zt[64 * wi:64 * (wi + 1), :],
                            in_=psZ[64 * wi:64 * (wi + 1), :])
                nc.sync.dma_start(
                    out=z_d[p, y0:y0 + 2].rearrange(
                        "y w1 w2 d -> (y w1 w2) d"), in_=zt)

            import contextlib
            loop_cm = (tc.For_i(0, loop, 1) if loop > 1
                       else contextlib.nullcontext())
            with loop_cm:
                # software-pipelined emission: skew FE/MID/BE across pairs
                states = {}
                for g in range(10):
                    if g < 8:
                        states[g] = frontend(g)
                    if 1 <= g <= 8:
                        midend(g - 1, states[g - 1])
                    if g >= 2:
                        backend(g - 2, states[g - 2])
                        del states[g - 2]

    if split:
        _split_multi_waits(nc.m)
    _BUILD_CACHE[key] = nc
    return nc


def kernel(q, k, v, ln_q_g, ln_q_b, ln_k_g, ln_k_b, ln_v_g, ln_v_b,
           Wq, bq, Wk, bk, Wv, bv, Wp, bp):
    q = np.asarray(q, np.float32)
    k = np.asarray(k, np.float32)
    v = np.asarray(v, np.float32)
    b, n, x, y, w1, w2, d = q.shape

    Wq_f = np.asarray(ln_q_g)[:, None] * np.asarray(Wq)
    Wq_p = np.zeros((128, 4, 128), np.float32)
    for h in range(4):
        Wq_p[:, h, 32 * h:32 * (h + 1)] = Wq_f[:, 32 * h:32 * (h + 1)]
    Wq_p = Wq_p.astype(ml_dtypes.bfloat16)
    Wk_e = (np.asarray(ln_k_g)[:, None] * np.asarray(Wk)).astype(ml_dtypes.bfloat16)
    Wv_e = (np.asarray(ln_v_g)[:, None] * np.asarray(Wv)).astype(ml_dtypes.bfloat16)
    Wp_e = (np.asarray(Wp) * 0.25).astype(ml_dtypes.bfloat16)
    cq = (np.asarray(ln_q_b) @ np.asarray(Wq) + np.asarray(bq)).astype(np.float32)
    ck = (np.asarray(ln_k_b) @ np.asarray(Wk) + np.asarray(bk)).astype(np.float32)
    cv = (np.asarray(ln_v_b) @ np.asarray(Wv) + np.asarray(bv)).astype(np.float32)
    bpv = np.asarray(bp, np.float32)

    add_cq, add_ck = bool(np.any(cq)), bool(np.any(ck))
    add_cv, add_bp = bool(np.any(cv)), bool(np.any(bpv))

    ident = np.eye(128, dtype=ml_dtypes.bfloat16)

    nc = _build_program(add_cq, add_ck, add_cv, add_bp)

    in_maps = []
    for core in range(8):
        pairs = [2 * core, 2 * core + 1]
        def shard(t):
            s = np.stack([t[pi // x, :, pi % x] for pi in pairs])
            return np.ascontiguousarray(s)
        m = {
            "xq": shard(q), "xk": shard(k), "xv": shard(v),
            "wqp": Wq_p, "wk": Wk_e, "wv": Wv_e, "wp": Wp_e,
            "ident": ident,
        }
        if add_cq:
            m["cq"] = cq[:, None]
        if add_ck:
            m["ck"] = ck[:, None]
        if add_cv:
            m["cv"] = cv[:, None]
        if add_bp:
            m["bpb"] = np.tile(bpv[None, :], (128, 1))
        in_maps.append(m)

    import os
    r = run_bass_kernel_spmd(nc, in_maps, list(range(8)),
                             trace=bool(os.environ.get("KERNEL_TRACE")))
    global LAST
    LAST = {"exec_time_ns": r.exec_time_ns, "profile_json": r.profile_json,
            "instructions_and_trace": r.instructions_and_trace}
    res = r.results

    out = np.zeros((b, x, y, w1, w2, d), np.float32)
    for core in range(8):
        zc = res[core]["z"]
        for j, pi in enumerate([2 * core, 2 * core + 1]):
            out[pi // x, pi % x] = zc[j]
    return out

